# revision 14
# baseline (speedup 1.0000x reference)
"""Multi-head attention (B=2, S=2048, D=1024, H=16) on 8 TRN2 NeuronCores.

Sharding: tensor-parallel over heads (2 heads per core) for QKV projections and
attention; AllToAll repartitions the context to token-parallel for the output
projection (avoids the expensive AllReduce).

Key compaction: masked key positions contribute exactly 0 to softmax numerator
and denominator (exp(-1e9) == 0 in fp32), so the host gathers only unmasked
key/value tokens (padded to a 128 multiple; pad columns get -1e9 bias), which
cuts K/V projection, logits, exp and PV work proportionally.

kernel(**inputs) takes the FULL inputs (as produced by setup_inputs()) and
returns the FULL [2, 2048, 1024] output.
"""
import math

import numpy as np

import concourse.mybir as mybir
import concourse.tile as tile
from concourse import bacc, bass_utils

# problem constants (hardcoded per contract)
B, S, D, H = 2, 2048, 1024, 16
T = B * S                 # 4096 flattened query tokens
DEPTH = D // H            # 64
N_CORES = 8
HL = 2 * DEPTH            # 128 local head dims per core (2 heads)
TCHUNK = T // N_CORES     # 512 tokens per core for the output projection
NDT = D // 128            # 8 contraction tiles of 128
NQB = S // 512            # 4 query blocks of 512 per batch

F32 = mybir.dt.float32
F32R = mybir.dt.float32r


def build_nc(kblocks: int = 16, collective: bool = True, num_devices: int = N_CORES,
             dump: bool = False,
             phases=("kproj", "vproj", "qproj", "attn", "a2a", "out"), reps: int = 1):
    """kblocks: compacted key blocks (of 128) per batch; 16 = uncompacted."""
    phases = set(phases)
    SK = kblocks * 128        # compacted keys per batch
    TK = B * SK               # total compacted key tokens
    nc = bacc.Bacc(
        "TRN2", target_bir_lowering=False, debug=False, num_devices=num_devices
    )

    # ---- I/O ----
    xqT = nc.dram_tensor("xqT", [D, T], F32R, kind="ExternalInput")   # query^T
    xkT = nc.dram_tensor("xkT", [D, TK], F32R, kind="ExternalInput")  # compacted key^T
    xvT = nc.dram_tensor("xvT", [D, TK], F32R, kind="ExternalInput")  # compacted value^T
    wq = nc.dram_tensor("wq", [D, HL], F32R, kind="ExternalInput")    # per-core slice
    wk = nc.dram_tensor("wk", [D, HL], F32R, kind="ExternalInput")
    wvp = nc.dram_tensor("wvp", [D, 256], F32R, kind="ExternalInput")  # wv zero-padded
    wo = nc.dram_tensor("wo", [D, D], F32R, kind="ExternalInput")     # full Wo
    bq = nc.dram_tensor("bq", [HL, 1], F32, kind="ExternalInput")
    bk = nc.dram_tensor("bk", [HL, 1], F32, kind="ExternalInput")
    bvb = nc.dram_tensor("bvb", [128, 256], F32, kind="ExternalInput")  # bv bcast
    bob = nc.dram_tensor("bob", [128, D], F32, kind="ExternalInput")    # bo bcast
    maskb = nc.dram_tensor("maskb", [128, B * kblocks], F32, kind="ExternalInput")
    out = nc.dram_tensor("out", [TCHUNK, D], F32, kind="ExternalOutput")

    if dump:
        d_qT = nc.dram_tensor("d_qT", [128, T], F32, kind="ExternalOutput")
        d_kT = nc.dram_tensor("d_kT", [128, TK], F32, kind="ExternalOutput")
        d_v = nc.dram_tensor("d_v", [128, TK // 128, 130], F32, kind="ExternalOutput")
        d_ctxT = nc.dram_tensor("d_ctxT", [128, T], F32, kind="ExternalOutput")

    a2a_in = nc.dram_tensor("a2a_in", [N_CORES, HL, TCHUNK], F32)
    a2a_out = nc.dram_tensor("a2a_out", [N_CORES, HL, TCHUNK], F32)

    with tile.TileContext(nc) as tc:
        with (
            tc.tile_pool(name="w", bufs=1) as wp,        # weights / constants
            tc.tile_pool(name="big", bufs=1) as bigp,    # persistent activations
            tc.tile_pool(name="io", bufs=6) as iop,      # streaming tiles
            tc.tile_pool(name="ps", bufs=1, space="PSUM") as ps,
        ):
            AFT = mybir.ActivationFunctionType
            # ---- constants needed early ----
            wq_sb = wp.tile([128, NDT, HL], F32R)
            wk_sb = wp.tile([128, NDT, HL], F32R)
            wv_sb = wp.tile([128, NDT, 256], F32R)
            nc.sync.dma_start(wk_sb[:], wk.ap().rearrange("(t p) m -> p t m", p=128))
            nc.sync.dma_start(wv_sb[:], wvp.ap().rearrange("(t p) m -> p t m", p=128))
            nc.sync.dma_start(wq_sb[:], wq.ap().rearrange("(t p) m -> p t m", p=128))
            bq_sb = wp.tile([HL, 1], F32)
            bk_sb = wp.tile([HL, 1], F32)
            bvb_sb = wp.tile([128, 256], F32)
            maskb_sb = wp.tile([128, B * kblocks], F32)
            nc.sync.dma_start(bq_sb[:], bq.ap())
            nc.sync.dma_start(bk_sb[:], bk.ap())
            nc.sync.dma_start(bvb_sb[:], bvb.ap())
            nc.sync.dma_start(maskb_sb[:], maskb.ap())

            # persistent per-core activations
            qT_sb = bigp.tile([128, T], F32R)      # \hat q^T (2 heads stacked)
            kT_sb = bigp.tile([128, TK], F32R)
            # v natural, TK/128 token-tiles: [v_h0 (64) | 1 | v_h1 (64) | 1]
            v_sb = bigp.tile([128, TK // 128, 130], F32R)
            ctxT_sb = bigp.tile([128, T], F32)     # normalized ctx^T (2 heads stacked)

            ones32 = wp.tile([128, TK // 128], F32)
            nc.vector.memset(ones32[:], 1.0)
            nc.sync.dma_start(v_sb[:, :, 64], ones32[:].bitcast(F32R))
            nc.sync.dma_start(v_sb[:, :, 129], ones32[:].bitcast(F32R))

            def emit_qkproj(cc, w, w_sb, b_sb, xT, outT):
                """project columns [cc, cc+w) of xT with w_sb -> outT (d-major)."""
                ps_k = ps.tile([128, 512], F32, tag="mm512", bufs=3, name="ps_k")
                for dt in range(NDT):
                    xk_t = iop.tile([128, 512], F32R, tag="xin", name="xk_t")
                    nc.sync.dma_start(
                        xk_t[:, 0:w], xT.ap()[dt * 128:(dt + 1) * 128, cc:cc + w]
                    )
                    nc.tensor.matmul(
                        ps_k[:, 0:w], w_sb[:, dt, :], xk_t[:, 0:w],
                        start=(dt == 0), stop=(dt == NDT - 1),
                    )
                nc.vector.tensor_scalar_add(outT[:, cc:cc + w], ps_k[:, 0:w], b_sb[:])

            def emit_vproj(cc, w):
                """value projection for compacted tokens [cc, cc+w) -> v_sb."""
                tt0 = cc // 128
                ntt = w // 128
                ps_v = [
                    ps.tile([128, 512], F32, tag="mm512", bufs=3, name="ps_v")
                    for _ in range((ntt + 1) // 2)
                ]
                xv_ts = []
                for dt in range(NDT):
                    xv_t = iop.tile([128, 512], F32R, tag="xv", bufs=11, name="xv_t")
                    nc.sync.dma_start(
                        xv_t[:, 0:w], xvT.ap()[dt * 128:(dt + 1) * 128, cc:cc + w]
                    )
                    xv_ts.append(xv_t)
                # one accumulation group at a time per PSUM bank: a mid-bank
                # start=True clears has_written for the whole bank
                for tt in range(ntt):
                    ps_t = ps_v[tt // 2]
                    col = (tt % 2) * 256
                    for dt in range(NDT):
                        nc.tensor.matmul(
                            ps_t[:, col:col + 256],
                            xv_ts[dt][:, tt * 128:(tt + 1) * 128],
                            wv_sb[:, dt, :],
                            start=(dt == 0), stop=(dt == NDT - 1),
                        )
                for pair, ps_t in enumerate(ps_v):
                    npair = min(2, ntt - 2 * pair)
                    pv = ps_t.rearrange("p (t n) -> p t n", t=2)
                    bb = bvb_sb.rearrange("p (t n) -> p t n", t=2)
                    for h in range(2):
                        nc.vector.tensor_add(
                            v_sb[:, tt0 + 2 * pair: tt0 + 2 * pair + npair,
                                 h * 65: h * 65 + 64],
                            pv[:, 0:npair, h * 64: h * 64 + 64],
                            bb[:, 0:npair, h * 64: h * 64 + 64],
                        )

            def emit_attn(b, qb):
                qc = b * S + qb * 512
                ps_c0 = ps.tile([65, 512], F32, tag="ctx", bufs=2, name="ps_c0")
                ps_c1 = ps.tile([65, 512], F32, tag="ctx", bufs=2, name="ps_c1")
                for kb in range(kblocks):
                    kc = b * SK + kb * 128   # column in kT_sb / tile in v_sb
                    kt = kc // 128
                    ps_l0 = ps.tile([128, 512], F32, tag="logit", bufs=3, name="ps_l0")
                    ps_l1 = ps.tile([128, 512], F32, tag="logit", bufs=3, name="ps_l1")
                    nc.tensor.matmul(
                        ps_l0[:], kT_sb[0:64, kc:kc + 128], qT_sb[0:64, qc:qc + 512]
                    )
                    nc.tensor.matmul(
                        ps_l1[:], kT_sb[64:128, kc:kc + 128],
                        qT_sb[64:128, qc:qc + 512],
                    )
                    e0 = iop.tile([128, 512], F32R, tag="exp", bufs=4, name="e0")
                    e1 = iop.tile([128, 512], F32R, tag="exp", bufs=4, name="e1")
                    mcol = b * kblocks + kb
                    nc.scalar.activation(
                        e0[:], ps_l0[:], AFT.Exp,
                        bias=maskb_sb[:, mcol:mcol + 1], scale=0.125,
                    )
                    nc.scalar.activation(
                        e1[:], ps_l1[:], AFT.Exp,
                        bias=maskb_sb[:, mcol:mcol + 1], scale=0.125,
                    )
                    nc.tensor.matmul(
                        ps_c0[:], v_sb[:, kt, 0:65], e0[:],
                        start=(kb == 0), stop=(kb == kblocks - 1),
                    )
                    nc.tensor.matmul(
                        ps_c1[:], v_sb[:, kt, 65:130], e1[:],
                        start=(kb == 0), stop=(kb == kblocks - 1),
                    )
                # epilogue: normalize by denominators (psum row 64)
                r0 = iop.tile([1, 512], F32, tag="r0", bufs=2, name="r0")
                r1 = iop.tile([1, 512], F32, tag="r1", bufs=2, name="r1")
                nc.vector.reciprocal(r0[:], ps_c0[64:65, :])
                nc.vector.reciprocal(r1[:], ps_c1[64:65, :])
                rec0 = iop.tile([64, 512], F32, tag="rec0", bufs=2, name="rec0")
                rec1 = iop.tile([64, 512], F32, tag="rec1", bufs=2, name="rec1")
                nc.gpsimd.partition_broadcast(rec0[:], r0[:])
                nc.gpsimd.partition_broadcast(rec1[:], r1[:])
                nc.vector.tensor_mul(
                    ctxT_sb[0:64, qc:qc + 512], ps_c0[0:64, :], rec0[:]
                )
                nc.vector.tensor_mul(
                    ctxT_sb[64:128, qc:qc + 512], ps_c1[0:64, :], rec1[:]
                )
                # eager A2A input staging for this finished chunk
                j = qc // TCHUNK
                if "a2a" in phases:
                    nc.sync.dma_start(
                        a2a_in.ap()[j], ctxT_sb[:, j * TCHUNK:(j + 1) * TCHUNK]
                    )

            kv_chunks = [(cc, min(512, TK - cc)) for cc in range(0, TK, 512)]
            # split K/V/Q work into a batch-0 prologue and an interleaved rest.
            # batch-0 attention needs K,V for keys [0, SK) and Q for [0, S).
            kv_pro = [(cc, w) for cc, w in kv_chunks if cc < SK]
            kv_rest = [(cc, w) for cc, w in kv_chunks if cc >= SK]
            q_pro = [(tb * 512, 512) for tb in range(NQB)]
            q_rest = [(S + tb * 512, 512) for tb in range(NQB)]

            for rep in range(reps):
                if "kproj" in phases:
                    for cc, w in kv_pro:
                        emit_qkproj(cc, w, wk_sb, bk_sb, xkT, kT_sb)
                if "vproj" in phases:
                    for cc, w in kv_pro:
                        emit_vproj(cc, w)
                if "qproj" in phases:
                    for cc, w in q_pro:
                        emit_qkproj(cc, w, wq_sb, bq_sb, xqT, qT_sb)
                # batch-0 attention interleaved with remaining projections
                rest = ([("k",) + c for c in kv_rest] + [("v",) + c for c in kv_rest]
                        + [("q",) + c for c in q_rest])
                nsl = (len(rest) + NQB - 1) // NQB
                for qb in range(NQB):
                    if "attn" in phases:
                        emit_attn(0, qb)
                    for item in rest[qb * nsl:(qb + 1) * nsl]:
                        kind, cc, w = item
                        if kind == "k" and "kproj" in phases:
                            emit_qkproj(cc, w, wk_sb, bk_sb, xkT, kT_sb)
                        elif kind == "v" and "vproj" in phases:
                            emit_vproj(cc, w)
                        elif kind == "q" and "qproj" in phases:
                            emit_qkproj(cc, w, wq_sb, bq_sb, xqT, qT_sb)
                if "attn" in phases:
                    for qb in range(NQB):
                        emit_attn(1, qb)

                if dump:
                    nc.sync.dma_start(d_qT.ap(), qT_sb[:].bitcast(F32))
                    nc.sync.dma_start(d_kT.ap(), kT_sb[:].bitcast(F32))
                    nc.sync.dma_start(d_v.ap(), v_sb[:].bitcast(F32))
                    nc.sync.dma_start(d_ctxT.ap(), ctxT_sb[:])

                # late constants for the output projection
                if rep == 0 and "out" in phases:
                    wo_sb = wp.tile([128, NDT, D], F32R)
                    bob_sb = wp.tile([128, D], F32)
                    nc.sync.dma_start(
                        wo_sb[:], wo.ap().rearrange("(t p) m -> p t m", p=128)
                    )
                    nc.sync.dma_start(bob_sb[:], bob.ap())

                if "a2a" in phases:
                    if collective:
                        nc.gpsimd.collective_compute(
                            "AllToAll",
                            mybir.AluOpType.bypass,
                            replica_groups=[list(range(N_CORES))],
                            ins=[a2a_in.ap().opt()],
                            outs=[a2a_out.ap().opt()],
                        )
                    else:  # single-core timing/simulation variant
                        for j in range(N_CORES):
                            nc.sync.dma_start(a2a_out.ap()[j], a2a_in.ap()[j])

                # ---- output projection for my 512-token chunk ----
                if "out" in phases:
                    ctxf_sb = bigp.tile([128, N_CORES, TCHUNK], F32R, tag="ctxf")
                    for i in range(N_CORES):
                        nc.sync.dma_start(
                            ctxf_sb[:, i, :], a2a_out.ap()[i].bitcast(F32R)
                        )
                    for tt in range(TCHUNK // 128):
                        ps_o0 = ps.tile([128, 512], F32, tag="mm512", bufs=3,
                                        name="ps_o0")
                        ps_o1 = ps.tile([128, 512], F32, tag="mm512", bufs=3,
                                        name="ps_o1")
                        for i in range(N_CORES):
                            lhs = ctxf_sb[:, i, tt * 128:(tt + 1) * 128]
                            nc.tensor.matmul(
                                ps_o0[:], lhs, wo_sb[:, i, 0:512],
                                start=(i == 0), stop=(i == N_CORES - 1),
                            )
                            nc.tensor.matmul(
                                ps_o1[:], lhs, wo_sb[:, i, 512:1024],
                                start=(i == 0), stop=(i == N_CORES - 1),
                            )
                        o0 = iop.tile([128, 512], F32, tag="osb", bufs=4, name="o0")
                        o1 = iop.tile([128, 512], F32, tag="osb", bufs=4, name="o1")
                        nc.vector.tensor_add(o0[:], ps_o0[:], bob_sb[:, 0:512])
                        nc.vector.tensor_add(o1[:], ps_o1[:], bob_sb[:, 512:1024])
                        nc.sync.dma_start(
                            out.ap()[tt * 128:(tt + 1) * 128, 0:512], o0[:]
                        )
                        nc.sync.dma_start(
                            out.ap()[tt * 128:(tt + 1) * 128, 512:1024], o1[:]
                        )

    nc.compile()
    return nc


_NC_CACHE = {}


def _get_nc(kblocks):
    if kblocks not in _NC_CACHE:
        _NC_CACHE[kblocks] = build_nc(kblocks=kblocks)
    return _NC_CACHE[kblocks]


# inputs identical on every core -> uploaded once and replicated by XLA
_REPLICATED = {"xqT", "xkT", "xvT", "wo", "bob", "maskb"}

_RUNNER_CACHE = {}


def _make_runner(nc):
    """Compile a shard_map-wrapped executor for `nc` once; returns
    run(in_maps) -> list of per-core output dicts."""
    import jax
    from jax.sharding import Mesh, NamedSharding, PartitionSpec as P
    from jax.experimental.shard_map import shard_map
    import concourse.bass2jax as b2j

    b2j.install_neuronx_cc_hook()
    in_names, out_names, out_avals = [], [], []
    for alloc in nc.m.functions[0].allocations:
        if not isinstance(alloc, mybir.MemoryLocationSet):
            continue
        name = alloc.memorylocations[0].name
        if alloc.kind == "ExternalInput":
            in_names.append(name)
        elif alloc.kind == "ExternalOutput":
            out_names.append(name)
            out_avals.append(
                jax.core.ShapedArray(
                    tuple(alloc.tensor_shape), mybir.dt.np(alloc.dtype)
                )
            )
    pid_name = nc.partition_id_tensor.name if nc.partition_id_tensor else None
    n_params = len(in_names)
    all_in_names = in_names + out_names

    def _body(*args):
        return tuple(
            b2j._bass_exec_p.bind(
                *args,
                out_avals=tuple(out_avals),
                in_names=tuple(all_in_names),
                out_names=tuple(out_names),
                lowering_input_output_aliases=(),
                sim_require_finite=True,
                sim_require_nnan=True,
                nc=nc,
            )
        )

    devices = jax.devices()[:N_CORES]
    mesh = Mesh(np.asarray(devices), ("core",))

    def spec_for(name):
        return P() if name in _REPLICATED else P("core")

    in_specs = tuple(spec_for(n) for n in in_names) + (P("core"),) * len(out_names)
    out_specs = (P("core"),) * len(out_names)
    fn = jax.jit(
        shard_map(_body, mesh=mesh, in_specs=in_specs, out_specs=out_specs,
                  check_rep=False),
        keep_unused=True,
    )
    sh_core = NamedSharding(mesh, P("core"))
    sh_repl = NamedSharding(mesh, P())
    zero_outs = [
        np.zeros((N_CORES * a.shape[0],) + tuple(a.shape[1:]), a.dtype)
        for a in out_avals
    ]
    upload_cache = {}

    def _put(name, arr, sh):
        import hashlib
        key = hashlib.blake2b(arr.tobytes(), digest_size=16).digest()
        hit = upload_cache.get(name)
        if hit is not None and hit[0] == key:
            return hit[1]
        buf = jax.device_put(arr, sh)
        upload_cache[name] = (key, buf)
        return buf

    def run(in_maps):
        args = []
        for name in in_names:
            if name == pid_name:
                cat = np.arange(N_CORES, dtype=np.uint32).reshape(N_CORES, 1)
                args.append(_put(name, cat, sh_core))
            elif name in _REPLICATED:
                args.append(_put(name, np.asarray(in_maps[0][name]), sh_repl))
            else:
                cat = np.concatenate(
                    [np.asarray(m[name]) for m in in_maps], axis=0
                )
                args.append(_put(name, cat, sh_core))
        for i, z in enumerate(zero_outs):
            args.append(_put(f"__zero{i}", z, sh_core))
        outs = fn(*args)
        jax.block_until_ready(outs)
        res = []
        for c in range(N_CORES):
            d = {}
            for i, name in enumerate(out_names):
                arr = np.asarray(outs[i])
                per = arr.shape[0] // N_CORES
                d[name] = arr[c * per:(c + 1) * per]
            res.append(d)
        return res

    return run


def _get_runner(kblocks):
    if kblocks not in _RUNNER_CACHE:
        _RUNNER_CACHE[kblocks] = _make_runner(_get_nc(kblocks))
    return _RUNNER_CACHE[kblocks]


def prepare_in_maps(kblocks, query, key, value, mask, Wq, bq, Wk, bk, Wv, bv, Wo, bo):
    SK = kblocks * 128
    m = np.asarray(mask, dtype=np.float32).reshape(B, S)
    key2 = np.asarray(key, dtype=np.float32).reshape(T, D)
    val2 = np.asarray(value, dtype=np.float32).reshape(T, D)

    rows = np.zeros(B * SK, np.int64)
    maskb = np.full((128 * kblocks, B), -1e9, np.float32)
    for b in range(B):
        idx = np.flatnonzero(m[b] == 0)
        n = len(idx)
        assert n <= SK, f"unmasked count {n} exceeds capacity {SK}"
        rows[b * SK: b * SK + n] = b * S + idx
        maskb[:n, b] = 0.0
    # maskb[p, b*kblocks+kb] with p = position within block kb
    maskb = np.ascontiguousarray(
        maskb.reshape(kblocks, 128, B).transpose(1, 2, 0).reshape(128, B * kblocks)
    )

    xqT = np.ascontiguousarray(np.asarray(query, np.float32).reshape(T, D).T)
    xkT = np.ascontiguousarray(key2[rows].T)
    xvT = np.ascontiguousarray(val2[rows].T)
    Wo_c = np.ascontiguousarray(Wo, dtype=np.float32)
    bob = np.ascontiguousarray(np.broadcast_to(bo, (128, D)), dtype=np.float32)

    in_maps = []
    for c in range(N_CORES):
        sl = slice(c * HL, (c + 1) * HL)
        wv_c = np.asarray(Wv[:, sl], dtype=np.float32)
        wvp = np.zeros((D, 256), np.float32)
        wvp[:, 0:HL] = wv_c
        bv_c = np.asarray(bv[sl], dtype=np.float32)
        bvb = np.ascontiguousarray(np.tile(bv_c, (128, 2)))
        in_maps.append(
            {
                "xqT": xqT, "xkT": xkT, "xvT": xvT,
                "wq": np.ascontiguousarray(Wq[:, sl], dtype=np.float32),
                "wk": np.ascontiguousarray(Wk[:, sl], dtype=np.float32),
                "wvp": wvp,
                "wo": Wo_c,
                "bq": np.ascontiguousarray(bq[sl], dtype=np.float32).reshape(HL, 1),
                "bk": np.ascontiguousarray(bk[sl], dtype=np.float32).reshape(HL, 1),
                "bvb": bvb,
                "bob": bob,
                "maskb": maskb,
            }
        )
    return in_maps


def _pick_kblocks(mask):
    m = np.asarray(mask).reshape(B, S)
    maxn = int((m == 0).sum(axis=1).max())
    return min(S // 128, math.ceil(maxn / 128) + 1)


def kernel(**inputs) -> np.ndarray:
    kblocks = _pick_kblocks(inputs["mask"])
    in_maps = prepare_in_maps(kblocks, **inputs)
    try:
        run = _get_runner(kblocks)
        results = run(in_maps)
    except Exception:
        # robust fallback: the stock SPMD runner
        res = bass_utils.run_bass_kernel_spmd(
            _get_nc(kblocks), in_maps, core_ids=list(range(N_CORES))
        )
        results = res.results
    out = np.concatenate([results[c]["out"] for c in range(N_CORES)], axis=0)
    return out.reshape(B, S, D)
